# revision 1
# baseline (speedup 1.0000x reference)
"""DeeperGCN layer as a Bass/Tile kernel for TRN2, 8-core SPMD.

Sharding: nodes are partitioned contiguously across 8 cores (dst-sharded).
Host pre-sorts edges by destination into per-(core, node-tile) padded bins,
so every core's segment-softmax reductions are fully local. The only
collective is one AllGather of the BN+ReLU'd node features (hn), which every
core needs as gather source for its edges' src nodes.

Per node-tile (128 dst nodes, capacity NCHUNK*128 edges):
  - gather hn[src] rows via one indirect DMA (offsets [128, NCHUNK])
  - msg = relu(hn_src + eattr) + EPS  (elementwise, edges on partitions)
  - ex = exp(t*msg), mex = msg*ex     (interleaved [ex|mex] per chunk)
  - indicator A[e, n] = (dst_slot[e] == n) built on-device vs iota
  - one matmul per 128-edge chunk accumulates [den|num] in PSUM:
      nd[n, 0:128] += A.T @ ex ; nd[n, 128:256] += A.T @ mex
  - agg = num/(den+1e-16); x_r = agg + hn_row; MLP (2 GEMMs + LayerNorm)
  - out = h + mlp_out
"""

import dataclasses
import numpy as np

import concourse.bass as bass
import concourse.bacc as bacc
import concourse.tile as tile
import concourse.mybir as mybir
from concourse.masks import make_identity

F32 = mybir.dt.float32
F32R = mybir.dt.float32r
BF16 = mybir.dt.bfloat16
I32 = mybir.dt.int32
I16 = mybir.dt.int16
AF = mybir.ActivationFunctionType
OP = mybir.AluOpType

EPS = 1e-7
BN_EPS = 1e-5
LN_EPS = 1e-5
DEN_EPS = 1e-16


@dataclasses.dataclass
class Cfg:
    n_cores: int = 8
    H: int = 128
    NT: int = 49          # node tiles per core
    C_LO: int = 12        # chunks of src<SPLIT edges per tile
    C_HI: int = 6         # chunks of src>=SPLIT edges per tile
    stream_bf16: bool = False   # eattr/hn-gather/exmex/indicator in bf16
    skip_collective: bool = False  # replace AllGather with local copy (TimelineSim)
    # how many iseq chunks run on DVE as one broadcast TT (rest: gp per-chunk TS)
    iseq_dve_chunks: int = 0
    # how many of the NCHUNK chunk-columns of the big adds run on gpsimd
    add_gp_chunks: int = 0
    mex_gp_chunks: int = 0
    relu_on: str = "act"      # "act" | "dve"
    ablate: str = ""          # comma list: gather,iseq,mm,elem,mlp,eadma
    iseq_dve_ts: bool = False  # DVE part uses per-chunk tensor_scalar
    n_queues: int = 1          # SWDGE queues for gather round-robin
    hw_repeat: int = 1        # repeat phase B (timing amplification)
    stream_bufs: int = 2
    small_bufs: int = 3
    copies_on: str = "act"    # "act" | "dve"
    ln_reduce: str = "act"    # "act" | "dve"
    apply_b1: bool = False
    apply_b2: bool = False
    apply_ln_affine: bool = False

    SPLIT: int = 32768

    @property
    def NCHUNK(self):
        return self.C_LO + self.C_HI

    @property
    def NPC(self):
        return self.NT * 128

    @property
    def NP(self):
        return self.NPC * self.n_cores

    @property
    def H2(self):
        return 2 * self.H

    @property
    def sdt(self):
        return BF16 if self.stream_bf16 else F32

    @property
    def mm_in(self):
        # dtype of matmul operand tiles (walrus requires producers to round)
        return BF16 if self.stream_bf16 else F32R


def build_gcn(cfg: Cfg):
    H, H2, NT, NCHUNK = cfg.H, cfg.H2, cfg.NT, cfg.NCHUNK
    NPC, NP = cfg.NPC, cfg.NP
    CE = NCHUNK * 128  # edge capacity per tile
    sdt = cfg.sdt

    nc = bacc.Bacc("TRN2", target_bir_lowering=False, debug=False,
                   num_devices=cfg.n_cores, num_swdge_queues=cfg.n_queues)

    # ---- I/O ----
    h_t = nc.dram_tensor("h_t", [H, NPC], F32, kind="ExternalInput").ap()
    h_rows = nc.dram_tensor("h_rows", [NPC, H], F32, kind="ExternalInput").ap()
    bnw = nc.dram_tensor("bnw", [H, 1], F32, kind="ExternalInput").ap()
    bnb = nc.dram_tensor("bnb", [H, 1], F32, kind="ExternalInput").ap()
    bnm = nc.dram_tensor("bnm", [H, 1], F32, kind="ExternalInput").ap()
    bnv = nc.dram_tensor("bnv", [H, 1], F32, kind="ExternalInput").ap()
    t_sc = nc.dram_tensor("t_sc", [1, 1], F32, kind="ExternalInput").ap()
    W1 = nc.dram_tensor("W1", [H, H2], F32, kind="ExternalInput").ap()
    W2 = nc.dram_tensor("W2", [H2, H], F32, kind="ExternalInput").ap()
    if cfg.apply_b1:
        b1 = nc.dram_tensor("b1", [1, H2], F32, kind="ExternalInput").ap()
    if cfg.apply_b2:
        b2 = nc.dram_tensor("b2", [1, H], F32, kind="ExternalInput").ap()
    if cfg.apply_ln_affine:
        lnw = nc.dram_tensor("lnw", [1, H2], F32, kind="ExternalInput").ap()
        lnb = nc.dram_tensor("lnb", [1, H2], F32, kind="ExternalInput").ap()
    gidx = nc.dram_tensor("gidx", [NT, 128, CE // 16], I16,
                          kind="ExternalInput").ap()
    dst_sl = nc.dram_tensor("dst_sl", [NT, 128, NCHUNK], F32,
                            kind="ExternalInput").ap()
    eattr = nc.dram_tensor("eattr", [NT, 128, CE], sdt,
                           kind="ExternalInput").ap()
    out = nc.dram_tensor("out", [NPC, H], F32, kind="ExternalOutput").ap()

    # internal DRAM
    hnb = nc.dram_tensor("hnb", [NPC, H], F32).ap()  # own slice, f32
    if cfg.stream_bf16:
        hnbg = nc.dram_tensor("hnbg", [NPC, H], sdt).ap()  # AG input
    else:
        hnbg = hnb
    hnf = nc.dram_tensor("hnf", [NP, H], sdt, addr_space="Shared").ap()

    with tile.TileContext(nc) as tc:
        with tc.tile_pool(name="const", bufs=1) as cpool, \
             tc.tile_pool(name="colv", bufs=1) as colp:
            # constants
            ident = cpool.tile([128, 128], F32)
            make_identity(nc, ident[:])
            iota = cpool.tile([128, CE], sdt)
            nc.gpsimd.iota(iota[:], pattern=[[0, NCHUNK], [1, 128]], base=0,
                           channel_multiplier=0,
                           allow_small_or_imprecise_dtypes=True)
            w1_st = cpool.tile([H, H2], F32, tag="w1_st")
            nc.sync.dma_start(w1_st[:], W1[:])
            w1_sb = cpool.tile([H, H2], F32R, tag="w1_sb")
            nc.scalar.copy(w1_sb[:], w1_st[:])
            w2_st = cpool.tile([H2 // 2, 2 * H], F32, tag="w2_st")
            nc.sync.dma_start(w2_st[:, 0:H], W2[0:H, :])
            nc.sync.dma_start(w2_st[:, H:2 * H], W2[H:H2, :])
            w2_sb = cpool.tile([H2 // 2, 2 * H], F32R, tag="w2_sb")
            nc.scalar.copy(w2_sb[:], w2_st[:])
            w2a_sb = w2_sb[:, 0:H]
            w2b_sb = w2_sb[:, H:2 * H]

            # column vectors
            bnw_c = colp.tile([H, 1], F32)
            nc.sync.dma_start(bnw_c[:], bnw[:])
            bnb_c = colp.tile([H, 1], F32)
            nc.sync.dma_start(bnb_c[:], bnb[:])
            bnm_c = colp.tile([H, 1], F32)
            nc.sync.dma_start(bnm_c[:], bnm[:])
            bnv_c = colp.tile([H, 1], F32)
            nc.sync.dma_start(bnv_c[:], bnv[:])
            t_c1 = colp.tile([1, 1], F32)
            nc.sync.dma_start(t_c1[:], t_sc[:])
            t_c = colp.tile([128, 1], F32)
            nc.gpsimd.partition_broadcast(t_c[:], t_c1[:])
            teps_c = colp.tile([128, 1], F32)
            nc.vector.tensor_scalar_mul(teps_c[:], t_c[:], float(EPS))
            # constant bias columns for ACT (non-Copy ACT needs AP bias)
            bneps_c = colp.tile([128, 1], F32)
            nc.gpsimd.memset(bneps_c[:], float(BN_EPS))
            lneps_c = colp.tile([128, 1], F32)
            nc.gpsimd.memset(lneps_c[:], float(LN_EPS))
            zeros_h2 = cpool.tile([128, H2], F32, tag="zeros_h2")
            nc.gpsimd.memset(zeros_h2[:], 0.0)
            # bn affine: a = bnw / sqrt(bnv + eps); c = bnb - bnm * a
            sd_c = colp.tile([H, 1], F32)
            nc.scalar.activation(sd_c[:], bnv_c[:], AF.Sqrt, bias=bneps_c[:])
            rs_c = colp.tile([H, 1], F32)
            nc.vector.reciprocal(rs_c[:], sd_c[:])
            a_c = colp.tile([H, 1], F32)
            nc.vector.tensor_mul(a_c[:], bnw_c[:], rs_c[:])
            ma_c = colp.tile([H, 1], F32)
            nc.vector.tensor_mul(ma_c[:], bnm_c[:], a_c[:])
            c_c = colp.tile([H, 1], F32)
            nc.vector.tensor_sub(c_c[:], bnb_c[:], ma_c[:])

            # broadcast rows for optional affine params
            if cfg.apply_b1 or cfg.apply_b2 or cfg.apply_ln_affine:
                ones_c = colp.tile([1, 128], F32)
                nc.gpsimd.memset(ones_c[:], 1.0)
            with tc.tile_pool(name="bc_ps", bufs=2, space="PSUM") as bcps:
                def bcast_row(dram_row, width, nm):
                    ps = bcps.tile([128, width], F32, tag=f"bc_{nm}")
                    row = colp.tile([1, width], F32, tag=f"bcrow_{nm}")
                    nc.sync.dma_start(row[:], dram_row)
                    sb = cpool.tile([128, width], F32, tag=f"bcsb_{nm}")
                    nc.tensor.matmul(ps[:], lhsT=ones_c[:], rhs=row[:],
                                     start=True, stop=True)
                    nc.scalar.copy(sb[:], ps[:])
                    return sb
                b1_b = bcast_row(b1[:], H2, "b1") if cfg.apply_b1 else None
                b2_b = bcast_row(b2[:], H, "b2") if cfg.apply_b2 else None
                lnw_b = (bcast_row(lnw[:], H2, "lnw")
                         if cfg.apply_ln_affine else None)
                lnb_b = (bcast_row(lnb[:], H2, "lnb")
                         if cfg.apply_ln_affine else None)

            # ---- phase A: hn slice = relu(a*h + c), transposed out ----
            NBLK = 4
            BW = (NT + NBLK - 1) // NBLK * 128  # block width in h_t cols
            with tc.tile_pool(name="pa", bufs=2) as pa, \
                 tc.tile_pool(name="pa_ps", bufs=4, space="PSUM") as paps:
                for blk in range(NBLK):
                    c0 = blk * BW
                    c1 = min(c0 + BW, NPC)
                    if c0 >= c1:
                        break
                    htt = pa.tile([128, BW], F32, tag="htt")
                    nc.sync.dma_start(htt[:, 0:c1 - c0], h_t[:, c0:c1])
                    hnt = pa.tile([128, BW], F32, tag="hnt")
                    nc.scalar.activation(hnt[:, 0:c1 - c0], htt[:, 0:c1 - c0],
                                         AF.Relu, scale=a_c[:], bias=c_c[:])
                    for tt in range(c0 // 128, c1 // 128):
                        lo = tt * 128 - c0
                        ps = paps.tile([128, 128], F32, tag="pa_ps")
                        nc.tensor.transpose(ps[:], hnt[:, lo:lo + 128],
                                            ident[:])
                        hnr = pa.tile([128, 128], F32, tag="hnr")
                        nc.scalar.copy(hnr[:], ps[:])
                        nc.scalar.dma_start(hnb[tt * 128:(tt + 1) * 128, :],
                                            hnr[:])
                        if cfg.stream_bf16:
                            hng = pa.tile([128, 128], sdt, tag="hng")
                            nc.vector.tensor_copy(hng[:], ps[:])
                            nc.scalar.dma_start(
                                hnbg[tt * 128:(tt + 1) * 128, :], hng[:])

            # ---- AllGather hn ----
            if cfg.skip_collective:
                # timing-only stand-in: copy own slice into hnf
                nc.sync.dma_start(hnf[0:NPC, :], hnbg[:])
            else:
                nc.gpsimd.collective_compute(
                    "AllGather",
                    OP.bypass,
                    ins=[hnbg[:]],
                    outs=[hnf[:]],
                    replica_groups=[list(range(cfg.n_cores))],
                )

            # ---- phase B: per node tile ----
            with tc.tile_pool(name="stream", bufs=cfg.stream_bufs) as sp, \
                 tc.tile_pool(name="small", bufs=cfg.small_bufs) as smp, \
                 tc.tile_pool(name="ps_nd", bufs=2, space="PSUM") as ps_nd, \
                 tc.tile_pool(name="ps_tr", bufs=2, space="PSUM") as ps_tr, \
                 tc.tile_pool(name="ps_y", bufs=2, space="PSUM") as ps_y:
              for rep in range(cfg.hw_repeat):
                for t in range(NT):
                    # inputs for this tile
                    dsl = smp.tile([128, NCHUNK], F32, tag="dsl")
                    nc.scalar.dma_start(dsl[:], dst_sl[t])
                    ixt = smp.tile([128, CE // 16], I16, tag="ixt")
                    nc.scalar.dma_start(ixt[:], gidx[t])
                    ea = sp.tile([128, CE], sdt, tag="ea")
                    if "eadma" not in cfg.ablate:
                        nc.sync.dma_start(ea[:], eattr[t])
                    hs = sp.tile([128, CE], sdt, tag="hs")
                    hs3 = hs[:].rearrange("p (j c) -> p j c", c=128)
                    MAXC = 8  # <=1024 idxs per call (SWDGE ring limit)
                    qn = [0]
                    def gather_calls(c0, c1, tab):
                        for a in range(c0, c1, MAXC):
                            b = min(a + MAXC, c1)
                            nc.gpsimd.dma_gather(
                                out_ap=hs3[:, a:b, :],
                                in_ap=tab,
                                idxs_ap=ixt[:, (a * 128) // 16:(b * 128) // 16],
                                num_idxs=(b - a) * 128,
                                num_idxs_reg=(b - a) * 128,
                                elem_size=H,
                                queue_num=(t + qn[0]) % cfg.n_queues,
                            )
                            qn[0] += 1
                    if "gather" not in cfg.ablate:
                        gather_calls(0, cfg.C_LO, hnf[:])
                        if cfg.C_HI:
                            gather_calls(cfg.C_LO, NCHUNK,
                                         hnf[cfg.SPLIT:NP, :])
                    else:
                        nc.sync.dma_start(hs[:], eattr[t])
                    # s = hs + ea
                    s = sp.tile([128, CE], sdt, tag="s")
                    noelem = "elem" in cfg.ablate
                    gp = 0 if noelem else cfg.add_gp_chunks * 128
                    if gp:
                        nc.gpsimd.tensor_add(s[:, CE - gp:], hs[:, CE - gp:],
                                             ea[:, CE - gp:])
                    if gp < CE and not noelem:
                        nc.vector.tensor_add(s[:, :CE - gp], hs[:, :CE - gp],
                                             ea[:, :CE - gp])
                    # r = relu(s)
                    r = sp.tile([128, CE], sdt, tag="r")
                    if noelem:
                        pass
                    elif cfg.relu_on == "dve":
                        nc.vector.tensor_scalar_max(r[:], s[:], 0.0)
                    else:
                        nc.scalar.activation(r[:], s[:], AF.Relu)
                    r3 = r[:].rearrange("p (j c) -> p j c", c=128)
                    # exmex: [ex_j | mex_j] interleaved per chunk
                    exmex = sp.tile([128, 2 * CE], cfg.mm_in, tag="exmex")
                    em3 = exmex[:].rearrange("p (j c) -> p j c", c=256)
                    ex_v = em3[:, :, 0:128]
                    mex_v = em3[:, :, 128:256]
                    if not noelem:
                        nc.scalar.activation(ex_v, r3, AF.Exp,
                                             scale=t_c[:], bias=teps_c[:])
                    mgp = 0 if noelem else cfg.mex_gp_chunks
                    if mgp:
                        nc.gpsimd.scalar_tensor_tensor(
                            mex_v[:, NCHUNK - mgp:, :],
                            r3[:, NCHUNK - mgp:, :], float(EPS),
                            ex_v[:, NCHUNK - mgp:, :], OP.add, OP.mult)
                    if mgp < NCHUNK and not noelem:
                        nc.vector.scalar_tensor_tensor(
                            mex_v[:, :NCHUNK - mgp, :],
                            r3[:, :NCHUNK - mgp, :], float(EPS),
                            ex_v[:, :NCHUNK - mgp, :], OP.add, OP.mult)
                    # indicator A[e, n] = (dst_slot[e] == iota_n)
                    A = sp.tile([128, CE], cfg.mm_in, tag="A")
                    A3 = A[:].rearrange("p (j c) -> p j c", c=128)
                    i3 = iota[:].rearrange("p (j c) -> p j c", c=128)
                    kd = min(cfg.iseq_dve_chunks, NCHUNK)
                    if "iseq" in cfg.ablate:
                        kd = NCHUNK + 1  # emit nothing
                        nc.gpsimd.memset(A[:], 0.0)
                    if kd and kd <= NCHUNK:
                        if cfg.iseq_dve_ts:
                            for j in range(kd):
                                nc.vector.tensor_scalar(
                                    A3[:, j, :], i3[:, j, :],
                                    dsl[:, j:j + 1], None, OP.is_equal)
                        else:
                            d_b = dsl[:, 0:kd].unsqueeze(2).to_broadcast(
                                [128, kd, 128])
                            nc.vector.tensor_tensor(A3[:, 0:kd, :],
                                                    i3[:, 0:kd, :],
                                                    d_b, OP.is_equal)
                    for j in range(kd, NCHUNK):
                        nc.gpsimd.tensor_scalar(A3[:, j, :], i3[:, j, :],
                                                dsl[:, j:j + 1], None,
                                                OP.is_equal)
                    # accumulate [den | num]
                    nd = ps_nd.tile([128, 256], F32, tag="nd")
                    if "mm" in cfg.ablate:
                        nc.tensor.matmul(nd[:], lhsT=A[:, 0:128],
                                         rhs=exmex[:, 0:256],
                                         start=True, stop=True)
                    else:
                     for j in range(NCHUNK):
                        nc.tensor.matmul(
                            nd[:],
                            lhsT=A[:, j * 128:(j + 1) * 128],
                            rhs=exmex[:, j * 256:(j + 1) * 256],
                            start=(j == 0), stop=(j == NCHUNK - 1),
                        )
                    # agg = num/(den + 1e-16); x_r = agg + hn_row
                    nomlp = "mlp" in cfg.ablate
                    d1 = smp.tile([128, 128], F32, tag="d1")
                    nc.vector.tensor_scalar_add(d1[:], nd[:, 0:128],
                                                float(DEN_EPS))
                    if nomlp:
                        osb0 = smp.tile([128, 128], F32, tag="osb")
                        nc.vector.tensor_copy(osb0[:], nd[:, 0:128])
                        nc.scalar.dma_start(out[t * 128:(t + 1) * 128, :],
                                            osb0[:])
                        continue
                    rden = smp.tile([128, 128], F32, tag="rden")
                    nc.vector.reciprocal(rden[:], d1[:])
                    hnrow = smp.tile([128, 128], F32, tag="hnrow")
                    nc.scalar.dma_start(hnrow[:], hnb[t * 128:(t + 1) * 128, :])
                    agg = smp.tile([128, 128], F32, tag="agg")
                    nc.vector.tensor_mul(agg[:], nd[:, 128:256], rden[:])
                    aggx = smp.tile([128, 128], F32, tag="aggx")
                    nc.vector.tensor_add(aggx[:], agg[:], hnrow[:])
                    # MLP
                    tps = ps_tr.tile([128, 128], F32, tag="tps")
                    nc.tensor.transpose(tps[:], aggx[:], ident[:])
                    aggxT = smp.tile([128, 128], F32R, tag="aggxT")
                    if cfg.copies_on == "dve":
                        nc.vector.tensor_copy(aggxT[:], tps[:])
                    else:
                        nc.scalar.copy(aggxT[:], tps[:])
                    y1 = ps_y.tile([128, H2], F32, tag="y1")
                    nc.tensor.matmul(y1[:], lhsT=aggxT[:], rhs=w1_sb[:],
                                     start=True, stop=True)
                    y1s = smp.tile([128, H2], F32, tag="y1s")
                    sums = smp.tile([128, 1], F32, tag="sums")
                    if cfg.ln_reduce == "dve":
                        nc.vector.tensor_tensor_reduce(
                            y1s[:], y1[:], zeros_h2[:], 1.0, 0.0,
                            OP.add, OP.add, accum_out=sums[:])
                    else:
                        nc.scalar.activation(y1s[:], y1[:], AF.Copy,
                                             accum_out=sums[:])
                    if cfg.apply_b1:
                        nc.vector.tensor_add(y1s[:], y1s[:], b1_b[:])
                        nc.vector.tensor_reduce(sums[:], y1s[:],
                                                mybir.AxisListType.X, OP.add)
                    sq = smp.tile([128, H2], F32, tag="sq")
                    sumsq = smp.tile([128, 1], F32, tag="sumsq")
                    src_for_sq = y1s if cfg.apply_b1 else y1
                    if cfg.ln_reduce == "dve":
                        nc.vector.tensor_tensor_reduce(
                            sq[:], y1s[:], y1s[:], 1.0, 0.0,
                            OP.mult, OP.add, accum_out=sumsq[:])
                    else:
                        nc.scalar.activation(sq[:], src_for_sq[:], AF.Square,
                                             accum_out=sumsq[:])
                    mu = smp.tile([128, 1], F32, tag="mu")
                    nc.vector.tensor_scalar_mul(mu[:], sums[:], 1.0 / H2)
                    msq = smp.tile([128, 1], F32, tag="msq")
                    nc.vector.tensor_mul(msq[:], mu[:], mu[:])
                    var = smp.tile([128, 1], F32, tag="var")
                    nc.vector.scalar_tensor_tensor(var[:], sumsq[:], 1.0 / H2,
                                                   msq[:], OP.mult, OP.subtract)
                    sdv = smp.tile([128, 1], F32, tag="sdv")
                    nc.scalar.activation(sdv[:], var[:], AF.Sqrt,
                                         bias=lneps_c[:])
                    rstd = smp.tile([128, 1], F32, tag="rstd")
                    nc.vector.reciprocal(rstd[:], sdv[:])
                    z = smp.tile([128, H2], F32, tag="z")
                    nc.vector.tensor_scalar(z[:], y1s[:], mu[:], rstd[:],
                                            OP.subtract, OP.mult)
                    if cfg.apply_ln_affine:
                        nc.vector.tensor_mul(z[:], z[:], lnw_b[:])
                        nc.vector.tensor_add(z[:], z[:], lnb_b[:])
                    yr = smp.tile([128, H2], F32, tag="yr")
                    nc.scalar.activation(yr[:], z[:], AF.Relu)
                    # transpose both halves for GEMM2
                    o_ps = ps_y.tile([128, H], F32, tag="o_ps")
                    for half in range(2):
                        tph = ps_tr.tile([128, 128], F32, tag="tps")
                        nc.tensor.transpose(
                            tph[:], yr[:, half * 128:(half + 1) * 128],
                            ident[:])
                        yT = smp.tile([128, 128], F32R, tag="yT")
                        if cfg.copies_on == "dve":
                            nc.vector.tensor_copy(yT[:], tph[:])
                        else:
                            nc.scalar.copy(yT[:], tph[:])
                        nc.tensor.matmul(
                            o_ps[:], lhsT=yT[:],
                            rhs=(w2a_sb if half == 0 else w2b_sb),
                            start=(half == 0), stop=(half == 1))
                    xrow = smp.tile([128, 128], F32, tag="xrow")
                    nc.scalar.dma_start(xrow[:], h_rows[t * 128:(t + 1) * 128, :])
                    osb = smp.tile([128, 128], F32, tag="osb")
                    nc.vector.tensor_add(osb[:], o_ps[:], xrow[:])
                    if cfg.apply_b2:
                        nc.vector.tensor_add(osb[:], osb[:], b2_b[:])
                    nc.scalar.dma_start(out[t * 128:(t + 1) * 128, :], osb[:])

    nc.compile()
    return nc


# ---------------- host-side prep ----------------

def host_prep(h, edge_index, edge_attr, bn_weight, bn_bias, bn_mean, bn_var,
              t, W1, b1, ln_weight, ln_bias, W2, b2, n_cores=8, split=32768):
    """Returns (cfg, in_maps, meta). Pure data movement + layout."""
    h = np.asarray(h, np.float32)
    edge_index = np.asarray(edge_index).astype(np.int64)
    edge_attr = np.asarray(edge_attr, np.float32)
    N, H = h.shape
    E = edge_index.shape[1]

    NT = int(np.ceil(N / (n_cores * 128)))
    NPC = NT * 128
    NP = NPC * n_cores

    src = edge_index[0]
    dst = edge_index[1]
    SPLIT = split
    core = dst // NPC
    tile_in_core = (dst % NPC) // 128
    slot = dst % 128
    gtile = core * NT + tile_in_core  # global tile id
    hi_flag = (src >= SPLIT).astype(np.int64)

    order = np.lexsort((src, hi_flag, gtile))
    src_s = src[order]
    gt_s = gtile[order]
    slot_s = slot[order]
    hi_s = hi_flag[order]

    n_tiles_all = n_cores * NT
    counts_lo = np.bincount(gt_s[hi_s == 0], minlength=n_tiles_all)
    counts_hi = np.bincount(gt_s[hi_s == 1], minlength=n_tiles_all)
    C_LO = max(1, int(np.ceil(counts_lo.max() / 128)))
    C_HI = int(np.ceil(counts_hi.max() / 128))
    NCHUNK = C_LO + C_HI
    CAP = NCHUNK * 128

    # logical position of each edge within its tile: lo edges from 0,
    # hi edges from C_LO*128
    starts_lo = np.zeros(n_tiles_all, np.int64)
    np.cumsum(counts_lo[:-1], out=starts_lo[1:])
    starts_hi = np.zeros(n_tiles_all, np.int64)
    np.cumsum(counts_hi[:-1], out=starts_hi[1:])
    # rank within (gtile, group): edges are sorted by (gtile, hi, src)
    grp = gt_s * 2 + hi_s
    grp_starts = np.zeros(2 * n_tiles_all, np.int64)
    cnt2 = np.bincount(grp, minlength=2 * n_tiles_all)
    np.cumsum(cnt2[:-1], out=grp_starts[1:])
    rank = np.arange(E, dtype=np.int64) - grp_starts[grp]
    logical = np.where(hi_s == 0, rank, C_LO * 128 + rank)

    p_idx = logical % 128
    j_idx = logical // 128
    apply_b1 = not np.allclose(np.asarray(b1), 0.0)
    apply_b2 = not np.allclose(np.asarray(b2), 0.0)
    apply_ln = not (np.allclose(np.asarray(ln_weight), 1.0)
                    and np.allclose(np.asarray(ln_bias), 0.0))

    cfg = Cfg(n_cores=n_cores, H=H, NT=NT, C_LO=C_LO, C_HI=C_HI, SPLIT=SPLIT,
              apply_b1=apply_b1, apply_b2=apply_b2, apply_ln_affine=apply_ln)

    sdt_np = np.dtype(np.float32)

    dst_pad = np.full((n_tiles_all, 128, NCHUNK), -1.0, np.float32)
    ea_pad = np.zeros((n_tiles_all, 128, NCHUNK, H), np.float32)
    dst_pad[gt_s, p_idx, j_idx] = slot_s.astype(np.float32)
    ea_pad[gt_s, p_idx, j_idx, :] = edge_attr[order]
    ea_pad = ea_pad.reshape(n_tiles_all, 128, NCHUNK * H)

    # int16 gather indices, wrapped in 16 partitions, replicated to 128
    gidx16 = np.zeros((n_tiles_all, 16, CAP // 16), np.int16)
    idx_val = np.where(hi_s == 0, src_s, src_s - SPLIT).astype(np.int16)
    gidx16[gt_s, logical % 16, logical // 16] = idx_val
    gidx = np.broadcast_to(
        gidx16[:, None, :, :], (n_tiles_all, 8, 16, CAP // 16)
    ).reshape(n_tiles_all, 128, CAP // 16)

    h_pad = np.zeros((NP, H), np.float32)
    h_pad[:N] = h

    com = dict(
        bnw=np.asarray(bn_weight, np.float32).reshape(H, 1),
        bnb=np.asarray(bn_bias, np.float32).reshape(H, 1),
        bnm=np.asarray(bn_mean, np.float32).reshape(H, 1),
        bnv=np.asarray(bn_var, np.float32).reshape(H, 1),
        t_sc=np.asarray(t, np.float32).reshape(1, 1),
        W1=np.asarray(W1, np.float32),
        W2=np.asarray(W2, np.float32),
    )
    if apply_b1:
        com["b1"] = np.asarray(b1, np.float32).reshape(1, 2 * H)
    if apply_b2:
        com["b2"] = np.asarray(b2, np.float32).reshape(1, H)
    if apply_ln:
        com["lnw"] = np.asarray(ln_weight, np.float32).reshape(1, 2 * H)
        com["lnb"] = np.asarray(ln_bias, np.float32).reshape(1, 2 * H)

    in_maps = []
    for c in range(n_cores):
        sl = slice(c * NPC, (c + 1) * NPC)
        m = dict(com)
        m["h_t"] = np.ascontiguousarray(h_pad[sl].T)
        m["h_rows"] = np.ascontiguousarray(h_pad[sl])
        m["gidx"] = np.ascontiguousarray(gidx[c * NT:(c + 1) * NT])
        m["dst_sl"] = np.ascontiguousarray(dst_pad[c * NT:(c + 1) * NT])
        m["eattr"] = np.ascontiguousarray(
            ea_pad[c * NT:(c + 1) * NT]).astype(sdt_np)
        in_maps.append(m)

    meta = dict(N=N, NPC=NPC)
    return cfg, in_maps, meta


try:
    import ml_dtypes
    ml_bf16 = ml_dtypes.bfloat16
except ImportError:
    ml_bf16 = np.float32


def assemble_output(results, meta):
    N, NPC = meta["N"], meta["NPC"]
    parts = [r["out"] for r in results]
    return np.concatenate(parts, axis=0)[:N].astype(np.float32)


# ---------------- harness entrypoint ----------------

def kernel(**inputs):
    """Full-input DeeperGCN layer on 8 NeuronCores; returns [N, H] float32."""
    import ml_dtypes
    cfg, in_maps, meta = host_prep(**{k: np.asarray(v)
                                      for k, v in inputs.items()}, n_cores=8)
    cfg.stream_bf16 = True
    cfg.iseq_dve_chunks = 99     # all indicator compares on DVE
    cfg.iseq_dve_ts = True       # per-chunk tensor_scalar form
    cfg.copies_on = "act"
    cfg.ln_reduce = "act"
    for m in in_maps:
        m["eattr"] = m["eattr"].astype(ml_dtypes.bfloat16)

    nc = build_gcn(cfg)

    from concourse.bass_utils import run_bass_kernel_spmd
    res = run_bass_kernel_spmd(nc, in_maps, core_ids=list(range(cfg.n_cores)))
    return assemble_output(res.results, meta)

